# revision 72
# baseline (speedup 1.0000x reference)
"""ART/Restormer window-attention block on 8 Trainium2 cores.

Sharding: data-parallel over image rows. Core c gets rows [c*32, (c+1)*32)
of the 256x256 token grid = 8192 contiguous tokens (32 complete 16x16
windows), so attention is fully core-local; small params replicated.

V2 design notes (vs the identity-matmul-bias baseline):
- The dynamic position-bias MLP runs on the HOST (numpy); each head's
  256x256 bias matrix is SVD-factored to rank 96 and fused into the QK
  matmul as 96 extra contraction rows riding the unused PE partitions
  (d_head=32, so K=32+96=128). Bias costs zero device time.
- Per-head q/k tiles ([q_h; W_h] / [k_h; U_h] stacked on partitions) are
  assembled with SBUF->SBUF shift DMAs from a 3-pass M=128 QKV output.
- x is host-shuffled to block-contiguous token order: 1 input DMA per
  block, 4 output DMAs per block (HWDGE dispatch is ~630ns each).
- proj and fc2 run token-major (activations as the stationary operand),
  which kills the output-side PE transposes and ACT identity copies.
- All matmuls bf16 (1 cycle/row); LN transposes f32r (1.5 c/row).
- One total-order chain on ACT ops keeps table loads at 2 per block
  (ln+exp share a table via the act-table patch; gelu is the other).
"""
import sys
import os
import numpy as np
import ml_dtypes

sys.path.insert(0, "/opt/trn_rl_repo")

import concourse.bass as bass
import concourse.tile as tile
from concourse import bacc, mybir, bass_utils
from concourse.tile import add_dep_helper

f32 = mybir.dt.float32
f32r = mybir.dt.float32r
bf16 = mybir.dt.bfloat16
AF = mybir.ActivationFunctionType
OP = mybir.AluOpType

DIM = 192
HEADS = 6
G = 16
DHEAD = 32
NCORES = 8
TOK = 8192
BTOK = 2048
NBLK = 4
NWIN = 8
RB = 96              # SVD rank of the fused position bias
SCALE = DHEAD ** -0.5
EPS = 1e-5


def _patch_act_tables():
    # Force ln+exp onto the combined natural_log_exp_and_others set by
    # emptying the exp-only and ln-only sets (indices preserved, so the
    # act_func_set_id still matches act_info.json for walrus).
    import concourse.bacc as _bacc
    if getattr(_bacc, "_act_tables_patched", False):
        return
    orig = _bacc.get_activation_tables

    def patched(arch):
        d = orig(arch)
        out = {}
        for name, fns in d.items():
            if name in ("exp_and_others", "natural_log"):
                out[name] = set()
            else:
                out[name] = fns
        return out

    _bacc.get_activation_tables = patched
    _bacc._act_tables_patched = True


def _build_program():
    _patch_act_tables()
    nc = bacc.Bacc("TRN2", target_bir_lowering=False, debug=False,
                   num_devices=NCORES)

    def inp(name, shape, dt=f32):
        return nc.dram_tensor(name, shape, dt, kind="ExternalInput")

    x_h = inp("x", [TOK, DIM])
    wqkv_h = inp("wqkv_aug", [193, 384], bf16)
    wv_h = inp("wv_aug", [193, DIM], bf16)
    wproj_h = inp("wproj_aug", [193, DIM], bf16)
    wfc1_h = inp("wfc1_aug", [193, 768], bf16)
    wfc2t_h = inp("wfc2t", [128, 6, DIM], bf16)
    fc2bb_h = inp("fc2bb", [128, DIM])
    posuw_h = inp("posuw", [12, RB, BTOK], bf16)
    ident_h = inp("ident", [128, 128])

    out_h = nc.dram_tensor("out", [TOK, DIM], f32, kind="ExternalOutput")

    with tile.TileContext(nc) as tc:
        _emit(nc, tc, locals())
    nc.compile()
    return nc


def _emit(nc, tc, H):
    x_h = H["x_h"]; out_h = H["out_h"]

    from contextlib import ExitStack
    ctx = ExitStack()
    with ctx:
        wp = ctx.enter_context(tc.tile_pool(name="weights", bufs=1))
        ps_t = ctx.enter_context(tc.tile_pool(name="ps_t", bufs=2, space="PSUM"))
        ps_mm = ctx.enter_context(tc.tile_pool(name="ps_mm", bufs=3, space="PSUM"))
        ps_at = ctx.enter_context(tc.tile_pool(name="ps_at", bufs=3, space="PSUM"))
        stgp = ctx.enter_context(tc.tile_pool(name="stg", bufs=1))
        fmA = ctx.enter_context(tc.tile_pool(name="fmA", bufs=3))
        blkp = ctx.enter_context(tc.tile_pool(name="blkp", bufs=1))
        xbp = ctx.enter_context(tc.tile_pool(name="xb", bufs=2))
        x1p = ctx.enter_context(tc.tile_pool(name="x1", bufs=18))
        xnp = ctx.enter_context(tc.tile_pool(name="xn", bufs=4))
        smallp = ctx.enter_context(tc.tile_pool(name="small", bufs=8))
        mvp = ctx.enter_context(tc.tile_pool(name="mv", bufs=40))
        vp = ctx.enter_context(tc.tile_pool(name="vaug", bufs=6))
        expp = ctx.enter_context(tc.tile_pool(name="expp", bufs=6))
        sep = ctx.enter_context(tc.tile_pool(name="sep", bufs=8))
        h1p = ctx.enter_context(tc.tile_pool(name="h1", bufs=1))
        otp = ctx.enter_context(tc.tile_pool(name="ot", bufs=2))

        # prefetch block 0's x ahead of the weight loads on the sync queue
        # so LN1 stats start immediately instead of behind ~13us of weights.
        xblk0 = xbp.tile([128, 16, DIM], f32, tag="xblk", name="xblk")
        for hf in range(2):
            nc.sync.dma_start(
                xblk0[:, hf * 8:(hf + 1) * 8, :],
                bass.AP(tensor=x_h, offset=hf * 1024 * DIM,
                        ap=[[DIM, 128], [128 * DIM, 8], [1, DIM]]))

        # ---------------- weights / constants ----------------
        def wload(h_, r0, r1, c1, name, eng=nc.sync):
            t = wp.tile([r1 - r0, c1], h_.dtype, tag=name, name=name)
            eng.dma_start(t[:], h_.ap()[r0:r1, 0:c1])
            return t
        wqkv_hi = wload(H["wqkv_h"], 0, 128, 384, "wqkv_hi")
        wqkv_lo = wload(H["wqkv_h"], 128, 193, 384, "wqkv_lo", nc.scalar)
        wv_hi = wload(H["wv_h"], 0, 128, DIM, "wv_hi")
        wv_lo = wload(H["wv_h"], 128, 193, DIM, "wv_lo", nc.scalar)
        wproj_hi = wload(H["wproj_h"], 0, 128, DIM, "wproj_hi")
        wproj_lo = wload(H["wproj_h"], 128, 193, DIM, "wproj_lo", nc.scalar)
        wfc1_hi = wload(H["wfc1_h"], 0, 128, 768, "wfc1_hi")
        wfc1_lo = wload(H["wfc1_h"], 128, 193, 768, "wfc1_lo", nc.scalar)
        wfc2t = wp.tile([128, 6, DIM], bf16, tag="wfc2t", name="wfc2t")
        nc.sync.dma_start(wfc2t[:], H["wfc2t_h"].ap())
        fc2bb = wp.tile([128, DIM], f32, tag="fc2bb", name="fc2bb")
        nc.scalar.dma_start(fc2bb[:], H["fc2bb_h"].ap())
        identR = wp.tile([128, 128], f32r, tag="identR", name="identR")
        nc.sync.dma_start(identR[:], H["ident_h"].ap().bitcast(f32r))
        eps_t = wp.tile([128, 1], f32, tag="eps", name="eps")
        nc.vector.memset(eps_t[:], EPS)
        ones1 = wp.tile([128, 1], f32, tag="ones1", name="ones1")
        nc.vector.memset(ones1[:], 1.0)

        # per-head q/k tiles: rows 0-31 data (DMA'd per block), rows 32-127
        # the rank-96 bias factors (loaded once).
        qt, kt = [], []
        for h in range(HEADS):
            kth = wp.tile([128, BTOK], bf16, tag=f"kt{h}", name=f"kt{h}")
            nc.sync.dma_start(
                kth[32:128, :],
                bass.AP(tensor=H["posuw_h"], offset=h * RB * BTOK,
                        ap=[[BTOK, RB], [1, BTOK]]))
            kt.append(kth)
            qth = wp.tile([128, BTOK], bf16, tag=f"qt{h}", name=f"qt{h}")
            nc.scalar.dma_start(
                qth[32:128, :],
                bass.AP(tensor=H["posuw_h"], offset=(6 + h) * RB * BTOK,
                        ap=[[BTOK, RB], [1, BTOK]]))
            qt.append(qth)

        # Pre-set constant regions of rotating buffers ONCE: the "ones" row
        # of the aoT lo segment (xnT writes re-assert it as the LN ones row,
        # so every fmA buffer keeps 1.0 there), and the ones columns of the
        # six rotating va buffers (attention identity copies never touch
        # cols 32:64). Removes per-block Pool memsets from the hot queue.
        for _ in range(3):
            fb = fmA.tile([128, 2 * BTOK], bf16, tag="fmA", name="fmA_init")
            nc.gpsimd.memset(fb[64:65, 2048:4096], 1.0)
        for _ in range(6):
            vb = vp.tile([128, 6, 64], bf16, tag="va", name="va_init")
            nc.gpsimd.memset(vb[:, :, 32:64], 1.0)

        # total-order chain for ACT ops: keeps the queue grouped by
        # activation-table family (2 table loads per block).
        prev_act = [None]

        def act_chain(ins_obj):
            if prev_act[0] is not None:
                add_dep_helper(ins_obj.ins, prev_act[0].ins, sync=False,
                               reason="act order")
            prev_act[0] = ins_obj
            return ins_obj

        def batch_rstd(var16, n):
            # exp/ln family; chained so they land exactly at the exps->gelus
            # boundary (before the gelu cluster), which unblocks the next
            # block's LN/transpose work during the gelu stretch.
            lnv = smallp.tile([128, n], f32, tag="lnv", name="lnv")
            act_chain(nc.scalar.activation(lnv[:], var16[:], AF.Ln,
                                           bias=eps_t[:]))
            rstd = smallp.tile([128, n], f32, tag="rstd", name="rstd")
            act_chain(nc.scalar.activation(rstd[:], lnv[:], AF.Exp,
                                           scale=-0.5))
            return rstd

        # ---------------- phases ----------------
        def load_xblk(blk):
            xblk = xbp.tile([128, 16, DIM], f32, tag="xblk", name="xblk")
            for hf in range(2):
                nc.sync.dma_start(
                    xblk[:, hf * 8:(hf + 1) * 8, :],
                    bass.AP(tensor=x_h,
                            offset=(blk * BTOK + hf * 1024) * DIM,
                            ap=[[DIM, 128], [128 * DIM, 8], [1, DIM]]))
            return xblk

        def emit_stats(blk, xblk=None):
            if xblk is None:
                xblk = load_xblk(blk)
            var16 = smallp.tile([128, 16], f32, tag="var16", name="var16")
            mvs = []
            for i in range(16):
                st = smallp.tile([128, 6], f32, tag="st", name="st")
                nc.vector.bn_stats(st[:], xblk[:, i, :])
                mv = mvp.tile([128, 2], f32, tag="mv", name="mv")
                nc.vector.bn_aggr(mv[:], st[:])
                nc.gpsimd.tensor_copy(var16[:, i:i + 1], mv[:, 1:2])
                mvs.append(mv)
            rstd = batch_rstd(var16, 16)
            return {"xblk": xblk, "mvs": mvs, "rstd": rstd}

        def norm_transpose_pair(srcs, mvs_, rcols, i0, dstT):
            # normalize two token-groups, transpose via PE (f32r), land both
            # hi segments and both lo segments with ONE batched DVE copy into
            # the unified feature-major tile (cols 0:2048 = features 0-127,
            # cols 2048:4096 = features 128-191 + ones row 64).
            tp = ps_t.tile([128, 512], f32r, tag="t")
            for a in range(2):
                i = i0 + a
                xn = xnp.tile([128, 256], f32r, tag="xn", name="xn")
                nc.gpsimd.tensor_scalar(xn[:, 0:DIM], srcs[a], mvs_[a][:, 0:1],
                                        rcols[a],
                                        op0=OP.subtract, op1=OP.mult)
                nc.gpsimd.tensor_copy(xn[:, DIM:DIM + 1], ones1[:])
                nc.tensor.transpose(tp[:, a * 128:(a + 1) * 128],
                                    xn[:, 0:128], identR[:])
                nc.tensor.transpose(tp[0:65, 256 + a * 128:256 + (a + 1) * 128],
                                    xn[:, 128:193], identR[:])
            d4 = dstT[:].rearrange("p (s g c) -> p s g c", s=2, c=128)
            nc.vector.tensor_copy(d4[:, :, i0:i0 + 2, :], tp[:].bitcast(f32))

        def emit_A(blk, S):
            xnT = fmA.tile([128, 2 * BTOK], bf16, tag="fmA", name="xnT")
            for i0 in range(0, 16, 2):
                rst = S["rstd"]
                norm_transpose_pair(
                    [S["xblk"][:, i0, :], S["xblk"][:, i0 + 1, :]],
                    S["mvs"][i0:i0 + 2],
                    [rst[:, i0:i0 + 1], rst[:, i0 + 1:i0 + 2]],
                    i0, xnT)
            stg = [stgp.tile([128, BTOK], bf16, tag=f"stg{m}", name=f"stg{m}")
                   for m in range(3)]
            for j in range(4):
                tsl = bass.ts(j, 512)
                for m in range(3):
                    pm = ps_mm.tile([128, 512], f32, tag="mm")
                    nc.tensor.matmul(pm[:], wqkv_hi[:, bass.ts(m, 128)],
                                     xnT[:, tsl], start=True, stop=False)
                    nc.tensor.matmul(pm[:], wqkv_lo[:, bass.ts(m, 128)],
                                     xnT[0:65, 2048 + j * 512:2048 + (j + 1) * 512],
                                     start=False, stop=True)
                    nc.scalar.activation(stg[m][:, tsl], pm[:], AF.Identity)
            for idx in range(12):
                m, grp = divmod(idx, 4)
                dst = qt[idx] if idx < 6 else kt[idx - 6]
                eng = nc.sync if idx % 2 == 0 else nc.scalar
                eng.dma_start(dst[0:32, :],
                              stg[m][grp * 32:(grp + 1) * 32, :])
            return {"xnT": xnT}

        def emit_B(blk, A, aoT, wr, mid_hook=None):
            xnT = A["xnT"]
            for w in wr:
                if w == 4 and mid_hook is not None:
                    mid_hook()
                vas = []
                for cv in range(2):
                    col = w * 256 + cv * 128
                    vps = ps_mm.tile([128, DIM], f32, tag="mm")
                    nc.tensor.matmul(vps[:], xnT[:, col:col + 128],
                                     wv_hi[:], start=True, stop=False)
                    nc.tensor.matmul(
                        vps[:], xnT[0:65, 2048 + col:2048 + col + 128],
                        wv_lo[:], start=False, stop=True)
                    va = vp.tile([128, 6, 64], bf16, tag="va", name="va")
                    nc.scalar.activation(
                        va[:, :, 0:32],
                        vps[:].rearrange("p (h d) -> p h d", h=6),
                        AF.Identity)
                    vas.append(va)
                for h in range(HEADS):
                    sp = ps_t.tile([128, 512], f32, tag="t")
                    for ck in range(2):
                        col = w * 256 + ck * 128
                        nc.tensor.matmul(sp[:, ck * 256:(ck + 1) * 256],
                                         kt[h][:, col:col + 128],
                                         qt[h][:, w * 256:(w + 1) * 256],
                                         start=True, stop=True)
                    e = expp.tile([128, 512], bf16, tag="ex", name="ex")
                    act_chain(nc.scalar.activation(e[:], sp[:], AF.Exp))
                    oa = ps_at.tile([64, 256], f32, tag="at")
                    for cv in range(2):
                        nc.tensor.matmul(oa[:], vas[cv][:, h, :],
                                         e[:, cv * 256:(cv + 1) * 256],
                                         start=(cv == 0), stop=(cv == 1))
                    oaS = sep.tile([64, 256], bf16, tag="oaS", name="oaS")
                    nc.vector.tensor_copy(oaS[:], oa[:])
                    se = sep.tile([32, 256], f32, tag="se", name="se")
                    nc.vector.reciprocal(se[:], oaS[32:64, :])
                    if h < 4:
                        dst = aoT[h * 32:(h + 1) * 32,
                                  w * 256:(w + 1) * 256]
                    else:
                        dst = aoT[(h - 4) * 32:(h - 3) * 32,
                                  2048 + w * 256:2048 + (w + 1) * 256]
                    nc.gpsimd.tensor_tensor(dst, oaS[0:32, :], se[:],
                                            op=OP.mult)

        def emit_p1(blk, aoT, S):
            xn2T = blkp.tile([128, 2 * BTOK], bf16, tag="xn2T", name="xn2T")
            var16 = smallp.tile([128, 16], f32, tag="var16b", name="var16b")
            x1s, mv2s, x1bs = [], [], []
            for g in range(16):
                pj = ps_mm.tile([128, DIM], f32, tag="mm")
                nc.tensor.matmul(pj[:], aoT[:, bass.ts(g, 128)],
                                 wproj_hi[:], start=True, stop=False)
                nc.tensor.matmul(
                    pj[:], aoT[0:65, 2048 + g * 128:2048 + (g + 1) * 128],
                    wproj_lo[:], start=False, stop=True)
                x1 = x1p.tile([128, DIM], f32, tag="x1", name="x1")
                nc.vector.tensor_tensor(x1[:], pj[:], S["xblk"][:, g, :],
                                        op=OP.add)
                st2 = smallp.tile([128, 6], f32, tag="st", name="st")
                nc.vector.bn_stats(st2[:], x1[:])
                mv2 = mvp.tile([128, 2], f32, tag="mv", name="mv")
                nc.vector.bn_aggr(mv2[:], st2[:])
                nc.gpsimd.tensor_copy(var16[:, g:g + 1], mv2[:, 1:2])
                # x1 + fc2 bias, overwriting the dead x slot (read in p2)
                x1b = S["xblk"][:, g, :]
                nc.gpsimd.tensor_tensor(x1b, x1[:], fc2bb[:], op=OP.add)
                x1s.append(x1); mv2s.append(mv2); x1bs.append(x1b)
            rstd2 = batch_rstd(var16, 16)
            for g0 in range(0, 16, 2):
                norm_transpose_pair(
                    [x1s[g0][:], x1s[g0 + 1][:]], mv2s[g0:g0 + 2],
                    [rstd2[:, g0:g0 + 1], rstd2[:, g0 + 1:g0 + 2]],
                    g0, xn2T)
            return {"xn2T": xn2T, "x1bs": x1bs}

        def emit_p2(blk, P1, jr):
            xn2T = P1["xn2T"]
            for j in jr:
                tsl = bass.ts(j, 512)
                f1s = []
                for m in range(6):
                    pool, tg = (ps_mm, "mm") if m % 2 == 0 else (ps_at, "at")
                    f1 = pool.tile([128, 512], f32, tag=tg)
                    nc.tensor.matmul(f1[:], wfc1_hi[:, bass.ts(m, 128)],
                                     xn2T[:, tsl], start=True, stop=False)
                    nc.tensor.matmul(f1[:], wfc1_lo[:, bass.ts(m, 128)],
                                     xn2T[0:65, 2048 + j * 512:2048 + (j + 1) * 512],
                                     start=False, stop=True)
                    f1s.append(f1)
                h1T = h1p.tile([128, 6, 512], bf16, tag="h1T", name="h1T")
                for m in range(6):
                    act_chain(nc.scalar.activation(h1T[:, m, :], f1s[m][:],
                                                   AF.Gelu))
                otj = otp.tile([128, 4, DIM], f32, tag="ot", name="ot")
                for g4 in range(4):
                    g = 4 * j + g4
                    fo = ps_mm.tile([128, DIM], f32, tag="mm")
                    for kc in range(6):
                        nc.tensor.matmul(fo[:],
                                         h1T[:, kc, g4 * 128:(g4 + 1) * 128],
                                         wfc2t[:, kc, :],
                                         start=(kc == 0), stop=(kc == 5))
                    nc.vector.tensor_tensor(otj[:, g4, :], fo[:],
                                            P1["x1bs"][g], op=OP.add)
                nc.sync.dma_start(
                    bass.AP(tensor=out_h, offset=(blk * BTOK + j * 512) * DIM,
                            ap=[[DIM, 128], [128 * DIM, 4], [1, DIM]]),
                    otj[:])

        # stats(b+1) is emitted mid-attention(b) so its rstd chains between
        # exp windows (exp-family, no table cost) and the whole next-block
        # prologue overlaps this block's attention + gelu stretch. (A(b+1)
        # must NOT be emitted mid-B: its shift-DMA WAR deps would only cover
        # already-emitted QK reads.)
        pending_p2 = None
        S = emit_stats(0, xblk0)
        for blk in range(NBLK):
            if pending_p2 is not None:
                emit_p2(blk - 1, pending_p2, range(0, 4))
            A = emit_A(blk, S)
            aoT = fmA.tile([128, 2 * BTOK], bf16, tag="fmA", name="aoT")
            S_next = [None]

            def hook(b=blk):
                if b + 1 < NBLK:
                    S_next[0] = emit_stats(b + 1)

            emit_B(blk, A, aoT, range(0, 8), mid_hook=hook)
            pending_p2 = emit_p1(blk, aoT, S)
            S = S_next[0]
        emit_p2(NBLK - 1, pending_p2, range(0, 4))


_NC = None


def _get_nc():
    global _NC
    if _NC is None:
        _NC = _build_program()
    return _NC


def _block_perm():
    # token order used on device: 4 blocks x (16 groups x 128 tokens),
    # group i of block b = image rows (b//2)*16 + (i%2)*8 .. +8,
    # cols (b%2)*128 + (i//2)*16 .. +16 (window-major within the group).
    perm = np.empty(TOK, np.int64)
    t = 0
    for b in range(NBLK):
        r0, c0 = (b // 2) * 16, (b % 2) * 128
        for i in range(16):
            for p in range(128):
                row = r0 + (i % 2) * 8 + p // 16
                col = c0 + (i // 2) * 16 + p % 16
                perm[t] = row * 256 + col
                t += 1
    return perm


_PERM = _block_perm()


def _host_inputs(inputs):
    d = {}
    g1 = np.asarray(inputs["gamma1"], np.float64)
    b1 = np.asarray(inputs["beta1"], np.float64)
    g2 = np.asarray(inputs["gamma2"], np.float64)
    b2 = np.asarray(inputs["beta2"], np.float64)
    qkv_w = np.asarray(inputs["qkv_w"], np.float64)
    qkv_b = np.asarray(inputs["qkv_b"], np.float64)
    wq = g1[:, None] * qkv_w
    bq = b1 @ qkv_w + qkv_b
    wq[:, 0:DIM] *= SCALE
    bq[0:DIM] *= SCALE
    wqkv_aug = np.zeros((193, 384), np.float32)
    wqkv_aug[0:DIM] = wq[:, 0:384]
    wqkv_aug[DIM] = bq[0:384]
    d["wqkv_aug"] = wqkv_aug.astype(ml_dtypes.bfloat16)
    wv_aug = np.zeros((193, DIM), np.float32)
    wv_aug[0:DIM] = wq[:, 384:576]
    wv_aug[DIM] = bq[384:576]
    d["wv_aug"] = wv_aug.astype(ml_dtypes.bfloat16)
    wproj_aug = np.zeros((193, DIM), np.float32)
    wproj_aug[0:DIM] = np.asarray(inputs["proj_w"], np.float32)
    wproj_aug[DIM] = np.asarray(inputs["proj_b"], np.float32)
    d["wproj_aug"] = wproj_aug.astype(ml_dtypes.bfloat16)
    fc1_w = np.asarray(inputs["fc1_w"], np.float64)
    fc1_b = np.asarray(inputs["fc1_b"], np.float64)
    wfc1_aug = np.zeros((193, 768), np.float32)
    wfc1_aug[0:DIM] = g2[:, None] * fc1_w
    wfc1_aug[DIM] = b2 @ fc1_w + fc1_b
    d["wfc1_aug"] = wfc1_aug.astype(ml_dtypes.bfloat16)
    wfc2 = np.asarray(inputs["fc2_w"], np.float32)
    d["wfc2t"] = np.ascontiguousarray(
        wfc2.reshape(6, 128, DIM).transpose(1, 0, 2)).astype(ml_dtypes.bfloat16)
    d["fc2bb"] = np.broadcast_to(
        np.asarray(inputs["fc2_b"], np.float32), (128, DIM)).copy()
    d["ident"] = np.eye(128, dtype=np.float32)

    # ---- position-bias MLP on host + SVD factorization -------------------
    def ln(x, g, b, eps=1e-5):
        m = x.mean(-1, keepdims=True)
        v = x.var(-1, keepdims=True)
        return (x - m) / np.sqrt(v + eps) * g + b

    r = np.arange(1 - G, G)
    grid = np.stack(np.meshgrid(r, r, indexing="ij")).reshape(2, -1).T
    p = grid.astype(np.float64) @ np.asarray(inputs["pos_proj_w"], np.float64) \
        + np.asarray(inputs["pos_proj_b"], np.float64)
    p = np.maximum(ln(p, inputs["ln1_g"], inputs["ln1_b"]), 0) \
        @ np.asarray(inputs["pos1_w"], np.float64) + np.asarray(inputs["pos1_b"], np.float64)
    p = np.maximum(ln(p, inputs["ln2_g"], inputs["ln2_b"]), 0) \
        @ np.asarray(inputs["pos2_w"], np.float64) + np.asarray(inputs["pos2_b"], np.float64)
    p = np.maximum(ln(p, inputs["ln3_g"], inputs["ln3_b"]), 0) \
        @ np.asarray(inputs["pos3_w"], np.float64) + np.asarray(inputs["pos3_b"], np.float64)
    c = np.stack(np.meshgrid(np.arange(G), np.arange(G), indexing="ij")).reshape(2, -1)
    rel = c[:, :, None] - c[:, None, :]
    idx = (rel[0] + G - 1) * (2 * G - 1) + (rel[1] + G - 1)   # [N, N]
    posuw = np.zeros((12, RB, BTOK), np.float32)
    for h in range(HEADS):
        Bh = p[idx, h]            # [n(query), m(key)]
        M = Bh.T                  # [key, query]
        U, Sv, Vt = np.linalg.svd(M)
        KU = (U[:, :RB] * np.sqrt(Sv[:RB])).T       # [RB, 256] key side
        QW = (np.sqrt(Sv[:RB])[:, None] * Vt[:RB])  # [RB, 256] query side
        posuw[h] = np.tile(KU.astype(np.float32), (1, NWIN))
        posuw[6 + h] = np.tile(QW.astype(np.float32), (1, NWIN))
    d["posuw"] = posuw.astype(ml_dtypes.bfloat16)
    return d


def kernel(**inputs):
    nc = _get_nc()
    x = np.asarray(inputs["x"], np.float32).reshape(65536, DIM)
    shared = _host_inputs(inputs)
    in_maps = []
    for c in range(NCORES):
        m = dict(shared)
        m["x"] = np.ascontiguousarray(x[c * TOK:(c + 1) * TOK][_PERM])
        in_maps.append(m)
    last_err = None
    for _ in range(3):
        try:
            res = bass_utils.run_bass_kernel_spmd(
                nc, in_maps, core_ids=list(range(NCORES)))
            break
        except Exception as e:  # transient NRT wedge after aborted runs
            last_err = e
            if "UNRECOVERABLE" not in repr(e) and "UNAVAILABLE" not in repr(e):
                raise
            os.environ["NEURON_RT_RESET_CORES"] = "1"
    else:
        raise last_err
    out = np.empty((65536, DIM), np.float32)
    for c in range(NCORES):
        out[c * TOK:(c + 1) * TOK][_PERM] = res.results[c]["out"]
    return out[None]


# revision 101
# speedup vs baseline: 1.0016x; 1.0016x over previous
"""ART/Restormer window-attention block on 8 Trainium2 cores.

Sharding: data-parallel over image rows. Core c gets rows [c*32, (c+1)*32)
of the 256x256 token grid = 8192 contiguous tokens (32 complete 16x16
windows), so attention is fully core-local; small params replicated.

V2 design notes (vs the identity-matmul-bias baseline):
- The dynamic position-bias MLP runs on the HOST (numpy); each head's
  256x256 bias matrix is SVD-factored to rank 96 and fused into the QK
  matmul as 96 extra contraction rows riding the unused PE partitions
  (d_head=32, so K=32+96=128). Bias costs zero device time.
- Per-head q/k tiles ([q_h; W_h] / [k_h; U_h] stacked on partitions) are
  assembled with SBUF->SBUF shift DMAs from a 3-pass M=128 QKV output.
- x is host-shuffled to block-contiguous token order: 1 input DMA per
  block, 4 output DMAs per block (HWDGE dispatch is ~630ns each).
- proj and fc2 run token-major (activations as the stationary operand),
  which kills the output-side PE transposes and ACT identity copies.
- All matmuls bf16 (1 cycle/row); LN transposes f32r (1.5 c/row).
- One total-order chain on ACT ops keeps table loads at 2 per block
  (ln+exp share a table via the act-table patch; gelu is the other).
"""
import sys
import os
import numpy as np
import ml_dtypes

sys.path.insert(0, "/opt/trn_rl_repo")

import concourse.bass as bass
import concourse.tile as tile
from concourse import bacc, mybir, bass_utils
from concourse.tile import add_dep_helper

f32 = mybir.dt.float32
f32r = mybir.dt.float32r
bf16 = mybir.dt.bfloat16
AF = mybir.ActivationFunctionType
OP = mybir.AluOpType

DIM = 192
HEADS = 6
G = 16
DHEAD = 32
NCORES = 8
TOK = 8192
BTOK = 2048
NBLK = 4
NWIN = 8
RB = 48              # SVD rank of the fused position bias
SCALE = DHEAD ** -0.5
EPS = 1e-5


def _patch_act_tables():
    # Force ln+exp onto the combined natural_log_exp_and_others set by
    # emptying the exp-only and ln-only sets (indices preserved, so the
    # act_func_set_id still matches act_info.json for walrus).
    import concourse.bacc as _bacc
    if getattr(_bacc, "_act_tables_patched", False):
        return
    orig = _bacc.get_activation_tables

    def patched(arch):
        d = orig(arch)
        out = {}
        for name, fns in d.items():
            if name in ("exp_and_others", "natural_log"):
                out[name] = set()
            else:
                out[name] = fns
        return out

    _bacc.get_activation_tables = patched
    _bacc._act_tables_patched = True


def _build_program():
    _patch_act_tables()
    nc = bacc.Bacc("TRN2", target_bir_lowering=False, debug=False,
                   num_devices=NCORES)

    def inp(name, shape, dt=f32):
        return nc.dram_tensor(name, shape, dt, kind="ExternalInput")

    x_h = inp("x", [TOK, DIM])
    wqkv_h = inp("wqkv_aug", [193, 384], bf16)
    wv_h = inp("wv_aug", [193, DIM], bf16)
    wproj_h = inp("wproj_aug", [193, DIM], bf16)
    wfc1_h = inp("wfc1_aug", [193, 768], bf16)
    wfc2t_h = inp("wfc2t", [128, 6, DIM], bf16)
    fc2bb_h = inp("fc2bb", [128, DIM])
    posuw_h = inp("posuw", [12, RB, BTOK], bf16)
    ident_h = inp("ident", [128, 128])

    out_h = nc.dram_tensor("out", [TOK, DIM], f32, kind="ExternalOutput")

    with tile.TileContext(nc) as tc:
        _emit(nc, tc, locals())
    nc.compile()
    return nc


def _emit(nc, tc, H):
    x_h = H["x_h"]; out_h = H["out_h"]

    from contextlib import ExitStack
    ctx = ExitStack()
    with ctx:
        wp = ctx.enter_context(tc.tile_pool(name="weights", bufs=1))
        ps_t = ctx.enter_context(tc.tile_pool(name="ps_t", bufs=2, space="PSUM"))
        ps_mm = ctx.enter_context(tc.tile_pool(name="ps_mm", bufs=3, space="PSUM"))
        ps_at = ctx.enter_context(tc.tile_pool(name="ps_at", bufs=3, space="PSUM"))
        stgp = ctx.enter_context(tc.tile_pool(name="stg", bufs=1))
        fmA = ctx.enter_context(tc.tile_pool(name="fmA", bufs=3))
        blkp = ctx.enter_context(tc.tile_pool(name="blkp", bufs=1))
        xbp = ctx.enter_context(tc.tile_pool(name="xb", bufs=2))
        x1p = ctx.enter_context(tc.tile_pool(name="x1", bufs=18))
        xnp = ctx.enter_context(tc.tile_pool(name="xn", bufs=4))
        smallp = ctx.enter_context(tc.tile_pool(name="small", bufs=8))
        mvp = ctx.enter_context(tc.tile_pool(name="mv", bufs=40))
        vp = ctx.enter_context(tc.tile_pool(name="vaug", bufs=6))
        expp = ctx.enter_context(tc.tile_pool(name="expp", bufs=6))
        sep = ctx.enter_context(tc.tile_pool(name="sep", bufs=8))
        h1p = ctx.enter_context(tc.tile_pool(name="h1", bufs=1))
        otp = ctx.enter_context(tc.tile_pool(name="ot", bufs=2))

        # prefetch block 0's x ahead of the weight loads on the sync queue
        # so LN1 stats start immediately instead of behind ~13us of weights.
        xblk0 = xbp.tile([128, 16, DIM], f32, tag="xblk", name="xblk")
        for hf in range(2):
            nc.sync.dma_start(
                xblk0[:, hf * 8:(hf + 1) * 8, :],
                bass.AP(tensor=x_h, offset=hf * 1024 * DIM,
                        ap=[[DIM, 128], [128 * DIM, 8], [1, DIM]]))

        # ---------------- weights / constants ----------------
        def wload(h_, r0, r1, c1, name, eng=nc.sync):
            t = wp.tile([r1 - r0, c1], h_.dtype, tag=name, name=name)
            eng.dma_start(t[:], h_.ap()[r0:r1, 0:c1])
            return t
        wqkv_hi = wload(H["wqkv_h"], 0, 128, 384, "wqkv_hi")
        wqkv_lo = wload(H["wqkv_h"], 128, 193, 384, "wqkv_lo", nc.scalar)
        wv_hi = wload(H["wv_h"], 0, 128, DIM, "wv_hi")
        wv_lo = wload(H["wv_h"], 128, 193, DIM, "wv_lo", nc.scalar)
        wproj_hi = wload(H["wproj_h"], 0, 128, DIM, "wproj_hi")
        wproj_lo = wload(H["wproj_h"], 128, 193, DIM, "wproj_lo", nc.scalar)
        wfc1_hi = wload(H["wfc1_h"], 0, 128, 768, "wfc1_hi")
        wfc1_lo = wload(H["wfc1_h"], 128, 193, 768, "wfc1_lo", nc.scalar)
        wfc2t = wp.tile([128, 6, DIM], bf16, tag="wfc2t", name="wfc2t")
        nc.sync.dma_start(wfc2t[:], H["wfc2t_h"].ap())
        fc2bb = wp.tile([128, DIM], f32, tag="fc2bb", name="fc2bb")
        nc.scalar.dma_start(fc2bb[:], H["fc2bb_h"].ap())
        identR = wp.tile([128, 128], f32r, tag="identR", name="identR")
        nc.sync.dma_start(identR[:], H["ident_h"].ap().bitcast(f32r))
        eps_t = wp.tile([128, 1], f32, tag="eps", name="eps")
        nc.vector.memset(eps_t[:], EPS)
        ones1 = wp.tile([128, 1], f32, tag="ones1", name="ones1")
        nc.vector.memset(ones1[:], 1.0)

        # per-head q/k tiles: rows 0-31 data (DMA'd per block), rows 32-127
        # the rank-96 bias factors (loaded once).
        qt, kt = [], []
        for h in range(HEADS):
            kth = wp.tile([128, BTOK], bf16, tag=f"kt{h}", name=f"kt{h}")
            nc.sync.dma_start(
                kth[32:32 + RB, :],
                bass.AP(tensor=H["posuw_h"], offset=h * RB * BTOK,
                        ap=[[BTOK, RB], [1, BTOK]]))
            kt.append(kth)
            qth = wp.tile([128, BTOK], bf16, tag=f"qt{h}", name=f"qt{h}")
            nc.scalar.dma_start(
                qth[32:32 + RB, :],
                bass.AP(tensor=H["posuw_h"], offset=(6 + h) * RB * BTOK,
                        ap=[[BTOK, RB], [1, BTOK]]))
            qt.append(qth)

        # Pre-set constant regions of rotating buffers ONCE: the "ones" row
        # of the aoT lo segment (xnT writes re-assert it as the LN ones row,
        # so every fmA buffer keeps 1.0 there), and the ones columns of the
        # six rotating va buffers (attention identity copies never touch
        # cols 32:64). Removes per-block Pool memsets from the hot queue.
        for _ in range(3):
            fb = fmA.tile([128, 2 * BTOK], bf16, tag="fmA", name="fmA_init")
            nc.gpsimd.memset(fb[64:65, 2048:4096], 1.0)
        for _ in range(6):
            vb = vp.tile([128, 6, 64], bf16, tag="va", name="va_init")
            nc.gpsimd.memset(vb[:, :, 32:64], 1.0)

        # total-order chain for ACT ops: keeps the queue grouped by
        # activation-table family (2 table loads per block).
        prev_act = [None]

        def act_chain(ins_obj):
            if prev_act[0] is not None:
                add_dep_helper(ins_obj.ins, prev_act[0].ins, sync=False,
                               reason="act order")
            prev_act[0] = ins_obj
            return ins_obj

        def batch_rstd(var16, n):
            # exp/ln family; chained so they land exactly at the exps->gelus
            # boundary (before the gelu cluster), which unblocks the next
            # block's LN/transpose work during the gelu stretch.
            lnv = smallp.tile([128, n], f32, tag="lnv", name="lnv")
            act_chain(nc.scalar.activation(lnv[:], var16[:], AF.Ln,
                                           bias=eps_t[:]))
            rstd = smallp.tile([128, n], f32, tag="rstd", name="rstd")
            act_chain(nc.scalar.activation(rstd[:], lnv[:], AF.Exp,
                                           scale=-0.5))
            return rstd

        # ---------------- phases ----------------
        def load_xblk(blk):
            xblk = xbp.tile([128, 16, DIM], f32, tag="xblk", name="xblk")
            for hf in range(2):
                nc.sync.dma_start(
                    xblk[:, hf * 8:(hf + 1) * 8, :],
                    bass.AP(tensor=x_h,
                            offset=(blk * BTOK + hf * 1024) * DIM,
                            ap=[[DIM, 128], [128 * DIM, 8], [1, DIM]]))
            return xblk

        def emit_stats(blk, xblk=None):
            if xblk is None:
                xblk = load_xblk(blk)
            var16 = smallp.tile([128, 16], f32, tag="var16", name="var16")
            mvs = []
            for i in range(16):
                st = smallp.tile([128, 6], f32, tag="st", name="st")
                nc.vector.bn_stats(st[:], xblk[:, i, :])
                mv = mvp.tile([128, 2], f32, tag="mv", name="mv")
                nc.vector.bn_aggr(mv[:], st[:])
                nc.gpsimd.tensor_copy(var16[:, i:i + 1], mv[:, 1:2])
                mvs.append(mv)
            rstd = batch_rstd(var16, 16)
            return {"xblk": xblk, "mvs": mvs, "rstd": rstd}

        def norm_transpose_pair(srcs, mvs_, rcols, i0, dstT):
            # normalize two token-groups, transpose via PE (f32r), land both
            # hi segments and both lo segments with ONE batched DVE copy into
            # the unified feature-major tile (cols 0:2048 = features 0-127,
            # cols 2048:4096 = features 128-191 + ones row 64).
            tp = ps_t.tile([128, 512], f32r, tag="t")
            for a in range(2):
                i = i0 + a
                xn = xnp.tile([128, 256], f32r, tag="xn", name="xn")
                nc.gpsimd.tensor_scalar(xn[:, 0:DIM], srcs[a], mvs_[a][:, 0:1],
                                        rcols[a],
                                        op0=OP.subtract, op1=OP.mult)
                nc.gpsimd.tensor_copy(xn[:, DIM:DIM + 1], ones1[:])
                nc.tensor.transpose(tp[:, a * 128:(a + 1) * 128],
                                    xn[:, 0:128], identR[:])
                nc.tensor.transpose(tp[0:65, 256 + a * 128:256 + (a + 1) * 128],
                                    xn[:, 128:193], identR[:])
            d4 = dstT[:].rearrange("p (s g c) -> p s g c", s=2, c=128)
            nc.vector.tensor_copy(d4[:, :, i0:i0 + 2, :], tp[:].bitcast(f32))

        def emit_A(blk, S):
            xnT = fmA.tile([128, 2 * BTOK], bf16, tag="fmA", name="xnT")
            for i0 in range(0, 16, 2):
                rst = S["rstd"]
                norm_transpose_pair(
                    [S["xblk"][:, i0, :], S["xblk"][:, i0 + 1, :]],
                    S["mvs"][i0:i0 + 2],
                    [rst[:, i0:i0 + 1], rst[:, i0 + 1:i0 + 2]],
                    i0, xnT)
            stg = [stgp.tile([128, BTOK], bf16, tag=f"stg{m}", name=f"stg{m}")
                   for m in range(3)]
            for j in range(4):
                tsl = bass.ts(j, 512)
                for m in range(3):
                    pm = ps_mm.tile([128, 512], f32, tag="mm")
                    nc.tensor.matmul(pm[:], wqkv_hi[:, bass.ts(m, 128)],
                                     xnT[:, tsl], start=True, stop=False)
                    nc.tensor.matmul(pm[:], wqkv_lo[:, bass.ts(m, 128)],
                                     xnT[0:65, 2048 + j * 512:2048 + (j + 1) * 512],
                                     start=False, stop=True)
                    nc.scalar.activation(stg[m][:, tsl], pm[:], AF.Identity)
            for idx in range(12):
                m, grp = divmod(idx, 4)
                dst = qt[idx] if idx < 6 else kt[idx - 6]
                eng = nc.sync if idx % 2 == 0 else nc.scalar
                eng.dma_start(dst[0:32, :],
                              stg[m][grp * 32:(grp + 1) * 32, :])
            return {"xnT": xnT}

        def emit_B(blk, A, aoT, wr, mid_hook=None):
            xnT = A["xnT"]
            for w in wr:
                if w == 4 and mid_hook is not None:
                    mid_hook()
                vas = []
                for cv in range(2):
                    col = w * 256 + cv * 128
                    vps = ps_mm.tile([128, DIM], f32, tag="mm")
                    nc.tensor.matmul(vps[:], xnT[:, col:col + 128],
                                     wv_hi[:], start=True, stop=False)
                    nc.tensor.matmul(
                        vps[:], xnT[0:65, 2048 + col:2048 + col + 128],
                        wv_lo[:], start=False, stop=True)
                    va = vp.tile([128, 6, 64], bf16, tag="va", name="va")
                    nc.scalar.activation(
                        va[:, :, 0:32],
                        vps[:].rearrange("p (h d) -> p h d", h=6),
                        AF.Identity)
                    vas.append(va)
                for h in range(HEADS):
                    sp = ps_t.tile([128, 512], f32, tag="t")
                    for ck in range(2):
                        col = w * 256 + ck * 128
                        nc.tensor.matmul(sp[:, ck * 256:(ck + 1) * 256],
                                         kt[h][0:32 + RB, col:col + 128],
                                         qt[h][0:32 + RB, w * 256:(w + 1) * 256],
                                         start=True, stop=True)
                    e = expp.tile([128, 512], bf16, tag="ex", name="ex")
                    act_chain(nc.scalar.activation(e[:], sp[:], AF.Exp))
                    oa = ps_at.tile([64, 256], f32, tag="at")
                    for cv in range(2):
                        nc.tensor.matmul(oa[:], vas[cv][:, h, :],
                                         e[:, cv * 256:(cv + 1) * 256],
                                         start=(cv == 0), stop=(cv == 1))
                    oaS = sep.tile([64, 256], bf16, tag="oaS", name="oaS")
                    nc.vector.tensor_copy(oaS[:], oa[:])
                    se = sep.tile([32, 256], f32, tag="se", name="se")
                    nc.vector.reciprocal(se[:], oaS[32:64, :])
                    if h < 4:
                        dst = aoT[h * 32:(h + 1) * 32,
                                  w * 256:(w + 1) * 256]
                    else:
                        dst = aoT[(h - 4) * 32:(h - 3) * 32,
                                  2048 + w * 256:2048 + (w + 1) * 256]
                    nc.gpsimd.tensor_tensor(dst, oaS[0:32, :], se[:],
                                            op=OP.mult)

        def emit_p1(blk, aoT, S):
            xn2T = blkp.tile([128, 2 * BTOK], bf16, tag="xn2T", name="xn2T")
            var16 = smallp.tile([128, 16], f32, tag="var16b", name="var16b")
            x1s, mv2s, x1bs = [], [], []
            for g in range(16):
                pj = ps_mm.tile([128, DIM], f32, tag="mm")
                nc.tensor.matmul(pj[:], aoT[:, bass.ts(g, 128)],
                                 wproj_hi[:], start=True, stop=False)
                nc.tensor.matmul(
                    pj[:], aoT[0:65, 2048 + g * 128:2048 + (g + 1) * 128],
                    wproj_lo[:], start=False, stop=True)
                x1 = x1p.tile([128, DIM], f32, tag="x1", name="x1")
                nc.vector.tensor_tensor(x1[:], pj[:], S["xblk"][:, g, :],
                                        op=OP.add)
                st2 = smallp.tile([128, 6], f32, tag="st", name="st")
                nc.vector.bn_stats(st2[:], x1[:])
                mv2 = mvp.tile([128, 2], f32, tag="mv", name="mv")
                nc.vector.bn_aggr(mv2[:], st2[:])
                nc.gpsimd.tensor_copy(var16[:, g:g + 1], mv2[:, 1:2])
                # x1 + fc2 bias, overwriting the dead x slot (read in p2)
                x1b = S["xblk"][:, g, :]
                nc.gpsimd.tensor_tensor(x1b, x1[:], fc2bb[:], op=OP.add)
                x1s.append(x1); mv2s.append(mv2); x1bs.append(x1b)
            rstd2 = batch_rstd(var16, 16)
            for g0 in range(0, 16, 2):
                norm_transpose_pair(
                    [x1s[g0][:], x1s[g0 + 1][:]], mv2s[g0:g0 + 2],
                    [rstd2[:, g0:g0 + 1], rstd2[:, g0 + 1:g0 + 2]],
                    g0, xn2T)
            return {"xn2T": xn2T, "x1bs": x1bs}

        def emit_p2(blk, P1, jr):
            xn2T = P1["xn2T"]
            for j in jr:
                tsl = bass.ts(j, 512)
                f1s = []
                for m in range(6):
                    pool, tg = (ps_mm, "mm") if m % 2 == 0 else (ps_at, "at")
                    f1 = pool.tile([128, 512], f32, tag=tg)
                    nc.tensor.matmul(f1[:], wfc1_hi[:, bass.ts(m, 128)],
                                     xn2T[:, tsl], start=True, stop=False)
                    nc.tensor.matmul(f1[:], wfc1_lo[:, bass.ts(m, 128)],
                                     xn2T[0:65, 2048 + j * 512:2048 + (j + 1) * 512],
                                     start=False, stop=True)
                    f1s.append(f1)
                h1T = h1p.tile([128, 6, 512], bf16, tag="h1T", name="h1T")
                for m in range(6):
                    act_chain(nc.scalar.activation(h1T[:, m, :], f1s[m][:],
                                                   AF.Gelu))
                otj = otp.tile([128, 4, DIM], f32, tag="ot", name="ot")
                for g4 in range(4):
                    g = 4 * j + g4
                    fo = ps_mm.tile([128, DIM], f32, tag="mm")
                    for kc in range(6):
                        nc.tensor.matmul(fo[:],
                                         h1T[:, kc, g4 * 128:(g4 + 1) * 128],
                                         wfc2t[:, kc, :],
                                         start=(kc == 0), stop=(kc == 5))
                    nc.vector.tensor_tensor(otj[:, g4, :], fo[:],
                                            P1["x1bs"][g], op=OP.add)
                nc.sync.dma_start(
                    bass.AP(tensor=out_h, offset=(blk * BTOK + j * 512) * DIM,
                            ap=[[DIM, 128], [128 * DIM, 4], [1, DIM]]),
                    otj[:])

        # stats(b+1) is emitted mid-attention(b) so its rstd chains between
        # exp windows (exp-family, no table cost) and the whole next-block
        # prologue overlaps this block's attention + gelu stretch. (A(b+1)
        # must NOT be emitted mid-B: its shift-DMA WAR deps would only cover
        # already-emitted QK reads.)
        pending_p2 = None
        S = emit_stats(0, xblk0)
        for blk in range(NBLK):
            if pending_p2 is not None:
                emit_p2(blk - 1, pending_p2, range(0, 4))
            A = emit_A(blk, S)
            aoT = fmA.tile([128, 2 * BTOK], bf16, tag="fmA", name="aoT")
            S_next = [None]

            def hook(b=blk):
                if b + 1 < NBLK:
                    S_next[0] = emit_stats(b + 1)

            emit_B(blk, A, aoT, range(0, 8), mid_hook=hook)
            pending_p2 = emit_p1(blk, aoT, S)
            S = S_next[0]
        emit_p2(NBLK - 1, pending_p2, range(0, 4))


_NC = None


def _get_nc():
    global _NC
    if _NC is None:
        _NC = _build_program()
    return _NC


def _block_perm():
    # token order used on device: 4 blocks x (16 groups x 128 tokens),
    # group i of block b = image rows (b//2)*16 + (i%2)*8 .. +8,
    # cols (b%2)*128 + (i//2)*16 .. +16 (window-major within the group).
    perm = np.empty(TOK, np.int64)
    t = 0
    for b in range(NBLK):
        r0, c0 = (b // 2) * 16, (b % 2) * 128
        for i in range(16):
            for p in range(128):
                row = r0 + (i % 2) * 8 + p // 16
                col = c0 + (i // 2) * 16 + p % 16
                perm[t] = row * 256 + col
                t += 1
    return perm


_PERM = _block_perm()


def _host_inputs(inputs):
    d = {}
    g1 = np.asarray(inputs["gamma1"], np.float64)
    b1 = np.asarray(inputs["beta1"], np.float64)
    g2 = np.asarray(inputs["gamma2"], np.float64)
    b2 = np.asarray(inputs["beta2"], np.float64)
    qkv_w = np.asarray(inputs["qkv_w"], np.float64)
    qkv_b = np.asarray(inputs["qkv_b"], np.float64)
    wq = g1[:, None] * qkv_w
    bq = b1 @ qkv_w + qkv_b
    wq[:, 0:DIM] *= SCALE
    bq[0:DIM] *= SCALE
    wqkv_aug = np.zeros((193, 384), np.float32)
    wqkv_aug[0:DIM] = wq[:, 0:384]
    wqkv_aug[DIM] = bq[0:384]
    d["wqkv_aug"] = wqkv_aug.astype(ml_dtypes.bfloat16)
    wv_aug = np.zeros((193, DIM), np.float32)
    wv_aug[0:DIM] = wq[:, 384:576]
    wv_aug[DIM] = bq[384:576]
    d["wv_aug"] = wv_aug.astype(ml_dtypes.bfloat16)
    wproj_aug = np.zeros((193, DIM), np.float32)
    wproj_aug[0:DIM] = np.asarray(inputs["proj_w"], np.float32)
    wproj_aug[DIM] = np.asarray(inputs["proj_b"], np.float32)
    d["wproj_aug"] = wproj_aug.astype(ml_dtypes.bfloat16)
    fc1_w = np.asarray(inputs["fc1_w"], np.float64)
    fc1_b = np.asarray(inputs["fc1_b"], np.float64)
    wfc1_aug = np.zeros((193, 768), np.float32)
    wfc1_aug[0:DIM] = g2[:, None] * fc1_w
    wfc1_aug[DIM] = b2 @ fc1_w + fc1_b
    d["wfc1_aug"] = wfc1_aug.astype(ml_dtypes.bfloat16)
    wfc2 = np.asarray(inputs["fc2_w"], np.float32)
    d["wfc2t"] = np.ascontiguousarray(
        wfc2.reshape(6, 128, DIM).transpose(1, 0, 2)).astype(ml_dtypes.bfloat16)
    d["fc2bb"] = np.broadcast_to(
        np.asarray(inputs["fc2_b"], np.float32), (128, DIM)).copy()
    d["ident"] = np.eye(128, dtype=np.float32)

    # ---- position-bias MLP on host + SVD factorization -------------------
    def ln(x, g, b, eps=1e-5):
        m = x.mean(-1, keepdims=True)
        v = x.var(-1, keepdims=True)
        return (x - m) / np.sqrt(v + eps) * g + b

    r = np.arange(1 - G, G)
    grid = np.stack(np.meshgrid(r, r, indexing="ij")).reshape(2, -1).T
    p = grid.astype(np.float64) @ np.asarray(inputs["pos_proj_w"], np.float64) \
        + np.asarray(inputs["pos_proj_b"], np.float64)
    p = np.maximum(ln(p, inputs["ln1_g"], inputs["ln1_b"]), 0) \
        @ np.asarray(inputs["pos1_w"], np.float64) + np.asarray(inputs["pos1_b"], np.float64)
    p = np.maximum(ln(p, inputs["ln2_g"], inputs["ln2_b"]), 0) \
        @ np.asarray(inputs["pos2_w"], np.float64) + np.asarray(inputs["pos2_b"], np.float64)
    p = np.maximum(ln(p, inputs["ln3_g"], inputs["ln3_b"]), 0) \
        @ np.asarray(inputs["pos3_w"], np.float64) + np.asarray(inputs["pos3_b"], np.float64)
    c = np.stack(np.meshgrid(np.arange(G), np.arange(G), indexing="ij")).reshape(2, -1)
    rel = c[:, :, None] - c[:, None, :]
    idx = (rel[0] + G - 1) * (2 * G - 1) + (rel[1] + G - 1)   # [N, N]
    posuw = np.zeros((12, RB, BTOK), np.float32)
    for h in range(HEADS):
        Bh = p[idx, h]            # [n(query), m(key)]
        M = Bh.T                  # [key, query]
        U, Sv, Vt = np.linalg.svd(M)
        KU = (U[:, :RB] * np.sqrt(Sv[:RB])).T       # [RB, 256] key side
        QW = (np.sqrt(Sv[:RB])[:, None] * Vt[:RB])  # [RB, 256] query side
        posuw[h] = np.tile(KU.astype(np.float32), (1, NWIN))
        posuw[6 + h] = np.tile(QW.astype(np.float32), (1, NWIN))
    d["posuw"] = posuw.astype(ml_dtypes.bfloat16)
    return d


def kernel(**inputs):
    nc = _get_nc()
    x = np.asarray(inputs["x"], np.float32).reshape(65536, DIM)
    shared = _host_inputs(inputs)
    in_maps = []
    for c in range(NCORES):
        m = dict(shared)
        m["x"] = np.ascontiguousarray(x[c * TOK:(c + 1) * TOK][_PERM])
        in_maps.append(m)
    last_err = None
    for _ in range(3):
        try:
            res = bass_utils.run_bass_kernel_spmd(
                nc, in_maps, core_ids=list(range(NCORES)))
            break
        except Exception as e:  # transient NRT wedge after aborted runs
            last_err = e
            if "UNRECOVERABLE" not in repr(e) and "UNAVAILABLE" not in repr(e):
                raise
            os.environ["NEURON_RT_RESET_CORES"] = "1"
    else:
        raise last_err
    out = np.empty((65536, DIM), np.float32)
    for c in range(NCORES):
        out[c * TOK:(c + 1) * TOK][_PERM] = res.results[c]["out"]
    return out[None]
